# revision 25
# baseline (speedup 1.0000x reference)
"""CenterLoss on 8 TRN2 NeuronCores (Bass kernel, data-parallel over batch).

Problem (fixed shapes, fp32):
    x       [4096, 2048]   features
    labels  [4096]         int    (class ids in [0, 6625))
    centers [6625, 2048]   class centers

    loss = mean_i( clip( ||x_i - centers[labels_i]||^2, 1e-12, 1e12 ) )

Sharding: batch split 512 rows/core across 8 cores; centers replicated
(stay in DRAM - only the 512 labeled rows are gathered per core).
Each core returns its partial sum of clamped squared distances; the
host sums the 8 partials and divides by 4096.

Inputs move as bf16 (host casts; halves DMA bytes, device accumulates
per-row sums in f32 - total loss error ~3e-6 relative).

Per-core pipeline (raw Bass, manual semaphores):
    sync    : labels DMA, then 4x 512KiB x-tile DMAs (HWDGE)
    gpsimd  : 4x indirect-DMA gathers centers[labels] -> SBUF (SWDGE)
    vector  : diff = x - c per [128,1024] chunk (bf16, 2x DVE mode) and
              square+accum for odd chunks (scalar_tensor_tensor)
    scalar  : square+accum for even chunks (Square activation accum_out)
    vector  : pair-combine + clamp [1e-12,1e12] -> [128,4]
    tensor  : ones[128,1].T @ dist4[128,4] -> per-tile sums in PSUM [1,4]
    scalar  : PSUM -> SBUF copy; sync DMAs the [1,1] partial out
"""

from contextlib import ExitStack

import ml_dtypes
import numpy as np

import concourse.bass as bass
import concourse.mybir as mybir
from concourse.bass_utils import run_bass_kernel_spmd

BATCH = 4096
FEAT = 2048
HALF = FEAT // 2
NCLASSES = 6625
NCORES = 8
SHARD = BATCH // NCORES  # 512 rows per core
P = 128                  # partitions
NT = SHARD // P          # 4 row-tiles of [128, FEAT] per core
NCHUNK = 2 * NT          # compute chunks of [128, HALF]
NG = NT + 1              # gathers: tile0 as two half-rows, tiles 1-3 full
NQ = 1                   # SWDGE queues (multi-queue routing is ignored by walrus)
NIW = SHARD // NT // 16  # idx columns per tile in the wrapped int16 layout (8)
F32 = mybir.dt.float32
BF16 = mybir.dt.bfloat16
I16 = mybir.dt.int16


def build_bass():
    nc = bass.Bass("TRN2", target_bir_lowering=False, debug=False,
                   num_swdge_queues=NQ)

    x = nc.dram_tensor("x", [SHARD, FEAT], BF16, kind="ExternalInput")
    # labels pre-arranged host-side to [128, NT]: labels_pn[p, n] = labels[n*128+p]
    labels = nc.dram_tensor("labels", [P, NT], mybir.dt.int32, kind="ExternalInput")
    centers = nc.dram_tensor("centers", [NCLASSES, FEAT], BF16, kind="ExternalInput")
    out = nc.dram_tensor("out", [1, 1], F32, kind="ExternalOutput")

    # chunk k covers row-tile n=k//2, feature half h=k%2
    def chunk_slice(k):
        n, h = divmod(k, 2)
        return slice(n * FEAT + h * HALF, n * FEAT + (h + 1) * HALF)

    DVE_SQ = (5, 7)  # odd late chunks' squares on DVE; chunk 6's square on
    # ACT so the final tile's two squares run on both engines in parallel

    # gather feeding chunk k: chunk 0 <- half-gather 0, chunk 1 <- half-
    # gather 1, chunks 2..7 <- full gathers 2..4 (tiles 1-3)
    def gather_of_chunk(k):
        return k if k < 2 else k // 2 + 1

    with ExitStack() as stack:
        sb = lambda *a: stack.enter_context(nc.sbuf_tensor(*a))
        sem = lambda name: stack.enter_context(nc.semaphore(name))

        xt = sb("xt", [P, NT * FEAT], BF16)
        ct = sb("ct", [P, NT * FEAT], BF16)
        diff = sb("diff", [P, NT * FEAT], BF16)
        scrq = sb("scrq", [P, len(DVE_SQ) * HALF], BF16)  # DVE square dumps
        lab = sb("lab", [P, NT], mybir.dt.int32)
        dist = sb("dist", [P, NCHUNK], F32)   # per-chunk row sums
        dist4 = sb("dist4", [P, NT], F32)     # per-row-tile sums
        warm = sb("warm", [P, 1], F32)
        dump4 = sb("dump4", [1, NT], F32)
        ones = sb("ones", [P, 1], F32)
        out_sb = sb("out_sb", [1, 1], F32)
        acc = stack.enter_context(nc.psum_tensor("acc", [1, NT], F32))

        labsem = sem("labsem")   # labels DMA
        outsem = sem("outsem")   # result DMA
        vsem = sem("vsem")       # every DVE data op
        asem = sem("asem")       # ACT square+accum ops
        vsem2 = sem("vsem2")     # DVE final chain done
        s1 = sem("s1")           # DVE self-sync
        s2 = sem("s2")           # DVE self-sync
        wsem = sem("wsem")       # warm buffer ready for ACT table warmup
        msem = sem("msem")       # PE matmul done
        osem = sem("osem")       # result in out_sb
        # one sem per DMA: concurrent DMAs on one sem can't be gated by
        # cumulative thresholds (per-engine completions interleave)
        xsem = [stack.enter_context(nc.semaphore(f"xsem{n}")) for n in range(NT)]
        csem = [stack.enter_context(nc.semaphore(f"csem{g}")) for g in range(NG)]
        block = stack.enter_context(nc.Block())

        @block.sync
        def _(sync):
            sync.dma_start(out=lab[:, :], in_=labels[:, :]).then_inc(labsem, 16)
            for n in range(NT):
                sync.dma_start(
                    out=xt[:, n * FEAT:(n + 1) * FEAT],
                    in_=x[n * P:(n + 1) * P, :],
                ).then_inc(xsem[n], 16)
            sync.wait_ge(osem, 1)
            # no wait on the out-DMA completion: NEFF teardown quiesces the
            # DMA rings, and skipping it lets the end barrier overlap the
            # ~1us HBM write receipt
            sync.dma_start(out=out[:, :], in_=out_sb[:, :]).then_inc(outsem, 16)

        @block.gpsimd
        def _(gpsimd):
            gpsimd.wait_ge(labsem, 16)  # labels landed
            # tile 0 as two half-row gathers so the compute pipeline starts
            # ~a gather earlier; tiles 1-3 as full-row gathers
            gspec = [(0, slice(0, HALF), 0), (0, slice(HALF, FEAT), HALF)]
            gspec += [(n, slice(n * FEAT, (n + 1) * FEAT), 0) for n in range(1, NT)]
            for g, (n, fsl, eoff) in enumerate(gspec):
                gpsimd.indirect_dma_start(
                    out=ct[:, fsl],
                    out_offset=None,
                    in_=centers[:, :],
                    in_offset=bass.IndirectOffsetOnAxis(ap=lab[:, n:n + 1], axis=0),
                    element_offset=eoff,
                ).then_inc(csem[g], 16)


        # DVE op positions for cross-engine waits: sub_k is followed by the
        # DVE square for odd k, every DVE data op bumps vsem by 1
        dve_pos = {}
        pos = 0
        for k in range(NCHUNK):
            pos += 1
            dve_pos[("sub", k)] = pos
            if k in DVE_SQ:
                pos += 1
                dve_pos[("sq", k)] = pos
        n_dve_ops = pos

        @block.vector
        def _(vector):
            vector.memset(warm[:, :], 1.0).then_inc(wsem, 1)
            vector.memset(ones[:, :], 1.0)
            for k in range(NCHUNK):
                fsl = chunk_slice(k)
                vector.wait_ge(xsem[k // 2], 16)
                vector.wait_ge(csem[gather_of_chunk(k)], 16)
                vector.tensor_sub(
                    out=diff[:, fsl], in0=xt[:, fsl], in1=ct[:, fsl]
                ).then_inc(vsem, 1)
                if k in DVE_SQ:
                    j = DVE_SQ.index(k)
                    vector.wait_ge(vsem, dve_pos[("sub", k)])  # diff_k retired
                    vector.scalar_tensor_tensor(
                        out=scrq[:, j * HALF:(j + 1) * HALF],
                        in0=diff[:, fsl], scalar=1.0, in1=diff[:, fsl],
                        op0=mybir.AluOpType.mult, op1=mybir.AluOpType.mult,
                        accum_out=dist[:, k:k + 1],
                    ).then_inc(vsem, 1)
            # combine half-chunk sums into per-row distances, clamp, reduce
            vector.wait_ge(asem, NCHUNK - len(DVE_SQ))
            vector.wait_ge(vsem, n_dve_ops)
            vector.tensor_add(
                out=dist4[:, :],
                in0=dist[:, 0:NCHUNK:2], in1=dist[:, 1:NCHUNK:2],
            ).then_inc(s1, 1)
            vector.wait_ge(s1, 1)
            vector.tensor_scalar(
                out=dist4[:, :], in0=dist4[:, :],
                scalar1=1e-12, scalar2=1e12,
                op0=mybir.AluOpType.max, op1=mybir.AluOpType.min,
            ).then_inc(vsem2, 1)

        @block.scalar
        def _(scalar):
            # dummy Square to pull the ACT PWP table load into the DMA phase
            scalar.wait_ge(wsem, 1)
            scalar.square(out=warm[:, :], in_=warm[:, :])
            for k in range(NCHUNK):
                if k in DVE_SQ:
                    continue
                fsl = chunk_slice(k)
                scalar.wait_ge(vsem, dve_pos[("sub", k)])
                scalar.activation(
                    out=diff[:, fsl], in_=diff[:, fsl],
                    func=mybir.ActivationFunctionType.Square,
                    accum_out=dist[:, k:k + 1],
                ).then_inc(asem, 1)
            scalar.wait_ge(msem, 1)
            # PSUM->SBUF copy fused with the final 4-wide sum via accum_out
            scalar.activation(
                out=dump4[:, :], in_=acc[:, :],
                func=mybir.ActivationFunctionType.Copy,
                accum_out=out_sb[:, :],
            ).then_inc(osem, 1)

        @block.tensor
        def _(tensor):
            tensor.wait_ge(vsem2, 1)
            tensor.matmul(
                out=acc[:, :], lhsT=ones[:, :], rhs=dist4[:, :],
                start=True, stop=True,
            ).then_inc(msem, 1)

    return nc


def make_in_maps(x, labels, centers):
    """Shard full inputs into per-core input maps (data-parallel over batch)."""
    # bf16 transport halves DMA traffic; squares/sums accumulate in f32 on
    # device, total loss error ~3e-6 relative - far inside tolerance
    x = np.ascontiguousarray(np.asarray(x, dtype=np.float32).astype(ml_dtypes.bfloat16))
    labels_i32 = np.asarray(labels).astype(np.int32)
    centers = np.ascontiguousarray(
        np.asarray(centers, dtype=np.float32).astype(ml_dtypes.bfloat16))
    assert x.shape == (BATCH, FEAT) and centers.shape == (NCLASSES, FEAT)
    assert labels_i32.shape == (BATCH,)
    return [
        {
            "x": x[c * SHARD:(c + 1) * SHARD],
            # [SHARD] -> [128, NT] with lab[p, n] = labels[n*128 + p]
            "labels": np.ascontiguousarray(
                labels_i32[c * SHARD:(c + 1) * SHARD].reshape(NT, P).T
            ),
            "centers": centers,
        }
        for c in range(NCORES)
    ]


def kernel(x, labels, centers):
    nc = build_bass()
    in_maps = make_in_maps(x, labels, centers)
    res = run_bass_kernel_spmd(nc, in_maps, core_ids=list(range(NCORES)))
    total = float(sum(float(r["out"].astype(np.float64).sum()) for r in res.results))
    return np.float32(total / BATCH)


if __name__ == "__main__":
    rng = np.random.default_rng(0)
    x = rng.standard_normal((BATCH, FEAT), dtype=np.float32)
    labels = rng.integers(0, NCLASSES, size=(BATCH,)).astype(np.int32)
    centers = rng.standard_normal((NCLASSES, FEAT), dtype=np.float32)
    got = kernel(x=x, labels=labels, centers=centers)
    c = centers[labels]
    d = ((x - c) ** 2).sum(axis=1)
    want = np.clip(d, 1e-12, 1e12).mean()
    print("kernel:", got, "numpy:", want, "rel:", abs(got - want) / abs(want))
